# revision 26
# baseline (speedup 1.0000x reference)
"""Causal multi-head self-attention with RoPE on 8 NeuronCores.

Sharding: batch (4) x head-halves (2) -> 8 cores. Core c handles batch c//2,
heads [8*(c%2), 8*(c%2)+8). Software-pipelined at (chunk, head-pair)
granularity: Q/K projection+RoPE units for the next head-pair are emitted one
iteration ahead of the attention that consumes them, so the tensor engine's
projection work overlaps the scalar engine's softmax exps. The output
projection is row-sharded over Wo with a pair ReduceScatter; each core emits
half of its pair's output rows.
"""

import numpy as np
import ml_dtypes

import concourse.bacc as bacc
import concourse.bass as bass
import concourse.mybir as mybir
from concourse.tile import TileContext
from concourse.bass_utils import run_bass_kernel_spmd

B, S, D, H = 4, 2048, 1024, 16
HL = 8          # heads per core
DK = 64         # head dim
NCORES = 8
DT = D // 128   # 8 d-tiles (contraction tiles)
OT = HL * DK // 128   # 4 o-tiles for Q^T/K^T (2 heads per tile)
ST = S // 128   # 16 s-tiles
NCH = S // 512  # 4 sequence chunks of 512
VW = DK + 1     # V columns per head incl. ones column

BF16 = mybir.dt.bfloat16
F32 = mybir.dt.float32
NEG = -1.0e9

_compiled = {}


def _build_nc():
    nc = bacc.Bacc("TRN2", target_bir_lowering=False, debug=False,
                   num_devices=NCORES)

    xT = nc.dram_tensor("xT", [D, S], BF16, kind="ExternalInput")
    wqT = nc.dram_tensor("wqT", [D, HL * DK], BF16, kind="ExternalInput")
    wkT = nc.dram_tensor("wkT", [D, HL * DK], BF16, kind="ExternalInput")
    wvT = nc.dram_tensor("wvT", [D, HL * DK], BF16, kind="ExternalInput")
    woT = nc.dram_tensor("woT", [HL * DK, D], BF16, kind="ExternalInput")
    cosT = nc.dram_tensor("cosT", [128, S], BF16, kind="ExternalInput")
    sinT = nc.dram_tensor("sinT", [128, S], BF16, kind="ExternalInput")
    swapT = nc.dram_tensor("swapT", [128, 128], BF16, kind="ExternalInput")
    maskT = nc.dram_tensor("maskT", [128, 128], F32, kind="ExternalInput")
    # each core outputs half of its pair's rows (ReduceScatter halves)
    y = nc.dram_tensor("y", [S // 2, D], BF16, kind="ExternalOutput")
    ypart = nc.dram_tensor("ypart", [S, D], BF16)
    yred = nc.dram_tensor("yred", [S // 2, D], BF16)

    groups = [[0, 1], [2, 3], [4, 5], [6, 7]]

    with TileContext(nc) as tc:
        with (
            tc.tile_pool(name="big", bufs=1) as big,
            tc.tile_pool(name="work", bufs=3) as work,
            tc.tile_pool(name="ptile", bufs=10) as ptile,
            tc.tile_pool(name="norm", bufs=2) as normp,
            tc.tile_pool(name="ps_s", bufs=2, space="PSUM") as ps_s,
            tc.tile_pool(name="ps_o", bufs=1, space="PSUM") as ps_o,
        ):
            # ---- inputs; ordered so chunk-0 work can start early ----
            w_sb = {"q": [], "k": [], "v": []}
            for k in range(DT):
                t = big.tile([128, HL * DK], BF16, tag=f"wq{k}", name=f"wq{k}")
                nc.scalar.dma_start(out=t[:], in_=wqT[128 * k:128 * (k + 1), :])
                w_sb["q"].append(t)
            mask_sb = big.tile([128, 128], F32, tag="mask")
            nc.scalar.dma_start(out=mask_sb[:], in_=maskT[:])
            cos_sb = big.tile([128, S], BF16, tag="cos")
            nc.scalar.dma_start(out=cos_sb[:, 0:512], in_=cosT[:, 0:512])
            sin_sb = big.tile([128, S], BF16, tag="sin")
            nc.scalar.dma_start(out=sin_sb[:, 0:512], in_=sinT[:, 0:512])
            swap_sb = big.tile([128, 128], BF16, tag="swap")
            nc.scalar.dma_start(out=swap_sb[:], in_=swapT[:])
            xT_sb = [big.tile([128, S], BF16, tag=f"xT{k}", name=f"xT{k}")
                     for k in range(DT)]
            for k in range(DT):
                nc.sync.dma_start(out=xT_sb[k][:, 0:512],
                                  in_=xT[128 * k:128 * (k + 1), 0:512])
            for k in range(DT):
                t = big.tile([128, HL * DK], BF16, tag=f"wk{k}", name=f"wk{k}")
                nc.gpsimd.dma_start(out=t[:], in_=wkT[128 * k:128 * (k + 1), :])
                w_sb["k"].append(t)
            for k in range(DT):
                t = big.tile([128, HL * DK], BF16, tag=f"wv{k}", name=f"wv{k}")
                nc.gpsimd.dma_start(out=t[:], in_=wvT[128 * k:128 * (k + 1), :])
                w_sb["v"].append(t)
            nc.gpsimd.dma_start(out=cos_sb[:, 512:1024], in_=cosT[:, 512:1024])
            nc.gpsimd.dma_start(out=sin_sb[:, 512:1024], in_=sinT[:, 512:1024])
            woT_sb = []
            for k in range(OT):
                t = big.tile([128, D], BF16, tag=f"wo{k}", name=f"wo{k}")
                nc.gpsimd.dma_start(out=t[:], in_=woT[128 * k:128 * (k + 1), :])
                woT_sb.append(t)
            nc.gpsimd.dma_start(out=cos_sb[:, 1024:S], in_=cosT[:, 1024:S])
            nc.gpsimd.dma_start(out=sin_sb[:, 1024:S], in_=sinT[:, 1024:S])

            # ---- persistent activations (per-chunk tiles) ----
            qrot_sb = [[big.tile([128, 512], BF16, tag=f"qr{m}_{t}",
                                 name=f"qr{m}_{t}") for t in range(OT)]
                       for m in range(NCH)]
            krot_sb = [[big.tile([128, 512], BF16, tag=f"kr{m}_{t}",
                                 name=f"kr{m}_{t}") for t in range(OT)]
                       for m in range(NCH)]
            vaug_sb = [big.tile([128, HL * VW], BF16, tag=f"va{st}",
                                name=f"va{st}") for st in range(ST)]
            for st in range(ST):
                ones = vaug_sb[st][:].rearrange(
                    "p (h d) -> p h d", d=VW)[:, :, DK:VW]
                nc.gpsimd.memset(ones, 1.0)
            oT_sb = [[big.tile([128, 512], BF16, tag=f"oT{m}_{t}",
                               name=f"oT{m}_{t}") for t in range(OT)]
                     for m in range(NCH)]

            def v_unit(m, st):
                ps = ps_s.tile([128, 1024], F32, tag="sps", name="psv")
                for k in range(DT):
                    nc.tensor.matmul(
                        ps[:, 0:512],
                        lhsT=xT_sb[k][:, 128 * st:128 * (st + 1)],
                        rhs=w_sb["v"][k][:],
                        start=(k == 0), stop=(k == DT - 1),
                    )
                dst = vaug_sb[st][:].rearrange(
                    "p (h d) -> p h d", d=VW)[:, :, 0:DK]
                src = ps[:, 0:512].rearrange("p (h d) -> p h d", d=DK)
                nc.vector.tensor_copy(dst, src)

            def qk_unit(wname, m, t):
                sl = slice(512 * m, 512 * (m + 1))
                rot = (qrot_sb if wname == "q" else krot_sb)[m][t]
                ps = ps_s.tile([128, 1024], F32, tag="sps", name="ps")
                for k in range(DT):
                    nc.tensor.matmul(
                        ps[:, 0:512],
                        lhsT=w_sb[wname][k][:, 128 * t:128 * (t + 1)],
                        rhs=xT_sb[k][:, sl],
                        start=(k == 0), stop=(k == DT - 1),
                    )
                raw = work.tile([128, 512], BF16, tag="raw")
                nc.vector.tensor_copy(raw[:], ps[:, 0:512])
                nc.tensor.matmul(ps[:, 512:1024], lhsT=swap_sb[:],
                                 rhs=raw[:], start=True, stop=True)
                t1 = work.tile([128, 512], BF16, tag="t1")
                nc.vector.tensor_mul(t1[:], raw[:], cos_sb[:, sl])
                t2 = work.tile([128, 512], BF16, tag="t2")
                nc.vector.tensor_mul(t2[:], ps[:, 512:1024], sin_sb[:, sl])
                nc.vector.tensor_add(rot[:], t1[:], t2[:])

            def attn_tp(m, tp, sums8, osb):
                i0 = 512 * m
                njb = 4 * m + 4
                o_ps4 = [[ps_o.tile([VW, 512], F32, tag=f"o{half}{jh}",
                                    bufs=1, name=f"o{half}{jh}")
                          for jh in range(2)] for half in range(2)]
                pTs = []
                for jb in range(njb):
                    j0 = 128 * jb
                    dlt = max(0, j0 - i0)
                    kr = krot_sb[jb // 4][tp]
                    jl = j0 % 512
                    s_ps = ps_s.tile([128, 1024], F32, tag="sps", name="sps")
                    for half, po in ((0, 0), (1, DK)):
                        nc.tensor.matmul(
                            s_ps[:, 512 * half + dlt:512 * (half + 1)],
                            lhsT=kr[po:po + DK, jl:jl + 128],
                            rhs=qrot_sb[m][tp][po:po + DK, dlt:512],
                            start=True, stop=True,
                        )
                    if j0 >= i0:
                        s3 = s_ps[:].rearrange("p (b f) -> p b f", b=2)
                        nc.vector.tensor_add(
                            s3[:, :, dlt:dlt + 128],
                            s3[:, :, dlt:dlt + 128],
                            mask_sb[:].rearrange("p (b f) -> p b f", b=1)
                            .broadcast_to([128, 2, 128]))
                    pT = ptile.tile([128, 1024], BF16, tag="pT")
                    nc.scalar.activation(
                        pT[:].rearrange("p (b f) -> p b f", b=2)[:, :, dlt:512],
                        s_ps[:].rearrange("p (b f) -> p b f", b=2)[:, :, dlt:512],
                        mybir.ActivationFunctionType.Exp, scale=0.125)
                    pTs.append(pT)
                for jb in range(njb):
                    dlt = max(0, 128 * jb - i0)
                    for half in range(2):
                        cs = VW * (2 * tp + half)
                        for jh in range(2):  # K-halves -> separate psum banks
                            nc.tensor.matmul(
                                o_ps4[half][jh][:, dlt:512],
                                lhsT=vaug_sb[jb][64 * jh:64 * (jh + 1),
                                                 cs:cs + VW],
                                rhs=pTs[jb][64 * jh:64 * (jh + 1),
                                            512 * half + dlt:512 * (half + 1)],
                                start=(jb == 0), stop=(jb == njb - 1),
                            )
                for half in range(2):
                    h = 2 * tp + half
                    ot = normp.tile([VW, 512], BF16, tag=f"osb{h}", bufs=1,
                                    name=f"osb{h}")
                    nc.vector.tensor_copy(ot[:], o_ps4[half][0][:])
                    nc.vector.tensor_add(ot[:], ot[:], o_ps4[half][1][:])
                    nc.sync.dma_start(out=sums8[tp // 2][h % 4:h % 4 + 1, :],
                                      in_=ot[DK:VW, :])
                    osb[h] = ot

            def norm_half(m, hf, sums8, osb):
                s32 = normp.tile([4, 512], F32, tag="s32", bufs=2, name="s32")
                nc.vector.tensor_copy(s32[:], sums8[hf][:])
                rec = normp.tile([4, 512], F32, tag="rec", bufs=2, name="rec")
                nc.vector.reciprocal_approx_fast(rec[:], s32[:])
                recb = normp.tile([4, 512], BF16, tag="recb", bufs=2,
                                  name="recb")
                nc.vector.tensor_copy(recb[:], rec[:])
                stage = normp.tile([1, 4 * 512], BF16, tag="stage", bufs=1,
                                   name="stage")
                nc.scalar.dma_start(out=stage[:], in_=recb[:])
                rep = normp.tile([DK, 4 * 512], BF16, tag="rep", bufs=1,
                                 name="rep")
                nc.gpsimd.partition_broadcast(rep[:], stage[:])
                for hh in range(4):
                    h = 4 * hf + hh
                    tp2, po = h // 2, DK * (h % 2)
                    nc.vector.tensor_mul(
                        oT_sb[m][tp2][po:po + DK, :], osb[h][0:DK, :],
                        rep[:, 512 * hh:512 * (hh + 1)])

            def proj_r(m, r2, ych_eng=None):
                r0 = 512 * m + 128 * r2
                yp = ps_s.tile([128, 1024], F32, tag="sps", name="yp")
                for nn in range(2):
                    for k in range(OT):
                        nc.tensor.matmul(
                            yp[:, 512 * nn:512 * (nn + 1)],
                            lhsT=oT_sb[m][k][:, 128 * r2:128 * (r2 + 1)],
                            rhs=woT_sb[k][:, 512 * nn:512 * (nn + 1)],
                            start=(k == 0), stop=(k == OT - 1),
                        )
                ych = work.tile([128, 1024], BF16, tag="ych")
                if ych_eng is None:
                    nc.vector.tensor_copy(ych[:], yp[:])
                else:
                    ych_eng.copy(out=ych[:], in_=yp[:])
                nc.sync.dma_start(out=ypart[r0:r0 + 128, :], in_=ych[:])

            def rs_chunk(m):
                q0 = 512 * m
                nc.gpsimd.collective_compute(
                    "ReduceScatter", mybir.AluOpType.add, replica_groups=groups,
                    ins=[ypart[q0:q0 + 512, :].opt()],
                    outs=[yred[256 * m:256 * (m + 1), :].opt()],
                )

            def y_dma(m):
                nc.sync.dma_start(out=y[256 * m:256 * (m + 1), :],
                                  in_=yred[256 * m:256 * (m + 1), :])

            # ---- pipelined emission ----
            # prep units per chunk: first two Q/K pairs, V units, last pairs
            prep = []
            for m in range(NCH):
                for t in (0, 1):
                    prep.append(("q", m, t))
                    prep.append(("k", m, t))
                for st in range(4 * m, 4 * m + 4):
                    prep.append(("v", m, st))
                for t in (2, 3):
                    prep.append(("q", m, t))
                    prep.append(("k", m, t))

            def emit_prep(upto):
                while emit_prep.cursor < min(upto, len(prep)):
                    kind, m, i = prep[emit_prep.cursor]
                    if kind == "v":
                        v_unit(m, i)
                    else:
                        qk_unit(kind, m, i)
                    emit_prep.cursor += 1
            emit_prep.cursor = 0

            def prefix_needed(m, tp):
                # attn(m, tp) needs all of chunk m's V plus Q/K pairs <= tp
                return 12 * m + {0: 8, 1: 8, 2: 10, 3: 12}[tp]

            sums8 = osb = None
            emit_prep(prefix_needed(0, 0))
            for m in range(NCH):
                sums8 = [normp.tile([4, 512], BF16, tag=f"sums{hf}", bufs=2,
                                    name=f"sums{hf}") for hf in range(2)]
                osb = [None] * HL
                for tp in range(OT):
                    attn_tp(m, tp, sums8, osb)
                    nxt = (m, tp + 1) if tp < 3 else (m + 1, 0)
                    if nxt[0] < NCH:
                        emit_prep(prefix_needed(*nxt))
                    if m >= 1 and tp < 2:
                        proj_r(m - 1, 2 * tp)
                        proj_r(m - 1, 2 * tp + 1)
                    if m >= 1 and tp == 2:
                        rs_chunk(m - 1)
                    if m == 0 and tp == 0:
                        for k in range(DT):
                            nc.sync.dma_start(
                                out=xT_sb[k][:, 512:1024],
                                in_=xT[128 * k:128 * (k + 1), 512:1024])
                    if m == 0 and tp == 2:
                        for k in range(DT):
                            nc.sync.dma_start(
                                out=xT_sb[k][:, 1024:S],
                                in_=xT[128 * k:128 * (k + 1), 1024:S])
                    if tp == 1:
                        norm_half(m, 0, sums8, osb)
                norm_half(m, 1, sums8, osb)
            for r2 in range(4):
                proj_r(NCH - 1, r2, ych_eng=nc.scalar)
            rs_chunk(NCH - 1)
            for m in range(NCH):
                y_dma(m)

    nc.compile()
    return nc


def _prep_inputs(x, Wq, Wk, Wv, Wo, cos_emb, sin_emb, token_positions):
    bf = ml_dtypes.bfloat16
    cos_g = np.asarray(cos_emb)[np.asarray(token_positions)]  # [S, DK]
    sin_g = np.asarray(sin_emb)[np.asarray(token_positions)]
    # [128, S]: partition p -> head-dim p % 64
    cosT = np.ascontiguousarray(np.tile(cos_g.T, (2, 1))).astype(bf)
    sinT = np.ascontiguousarray(np.tile(sin_g.T, (2, 1))).astype(bf)
    # rotate-half-interleaved as a matmul: rh = SWAP @ q (per 128-dim tile)
    swap = np.zeros((128, 128), np.float32)
    for j in range(64):
        swap[2 * j, 2 * j + 1] = -1.0
        swap[2 * j + 1, 2 * j] = 1.0
    swapT = np.ascontiguousarray(swap.T).astype(bf)
    # causal mask for the diagonal 128x128 block in S^T=[j,i] layout
    jj = np.arange(128)[:, None]
    ii = np.arange(128)[None, :]
    maskT = np.where(ii >= jj, 0.0, NEG).astype(np.float32)

    in_maps = []
    for c in range(NCORES):
        b, hh = c // 2, c % 2
        cols = slice(512 * hh, 512 * (hh + 1))
        in_maps.append({
            "xT": np.ascontiguousarray(np.asarray(x)[b].T).astype(bf),
            "wqT": np.ascontiguousarray(np.asarray(Wq)[cols, :].T).astype(bf),
            "wkT": np.ascontiguousarray(np.asarray(Wk)[cols, :].T).astype(bf),
            "wvT": np.ascontiguousarray(np.asarray(Wv)[cols, :].T).astype(bf),
            "woT": np.ascontiguousarray(np.asarray(Wo)[:, cols].T).astype(bf),
            "cosT": cosT, "sinT": sinT, "swapT": swapT, "maskT": maskT,
        })
    return in_maps


def kernel(x, Wq, Wk, Wv, Wo, cos_emb, sin_emb, token_positions, **run_kwargs):
    if "nc" not in _compiled:
        _compiled["nc"] = _build_nc()
    nc = _compiled["nc"]
    in_maps = _prep_inputs(x, Wq, Wk, Wv, Wo, cos_emb, sin_emb, token_positions)
    res = run_bass_kernel_spmd(nc, in_maps, list(range(NCORES)), **run_kwargs)
    # reassemble: per pair, chunk m rows [512m:512m+256) from the even core,
    # [512m+256:512m+512) from the odd core (ReduceScatter rank order)
    out = np.empty((B, S, D), np.float32)
    for b in range(B):
        for r in range(2):
            yh = np.asarray(res.results[2 * b + r]["y"]).astype(np.float32)
            for m in range(NCH):
                out[b, 512 * m + 256 * r:512 * m + 256 * (r + 1)] = \
                    yh[256 * m:256 * (m + 1)]
    if run_kwargs:
        kernel.last_result = res
    return out


# revision 27
# speedup vs baseline: 1.1561x; 1.1561x over previous
"""Causal multi-head self-attention with RoPE on 8 NeuronCores.

Sharding: batch (4) x head-halves (2) -> 8 cores. Core c handles batch c//2,
heads [8*(c%2), 8*(c%2)+8). Software-pipelined at (chunk, head-pair)
granularity: Q/K projection+RoPE units for the next head-pair are emitted one
iteration ahead of the attention that consumes them, so the tensor engine's
projection work overlaps the scalar engine's softmax exps. The output
projection is row-sharded over Wo with a pair ReduceScatter; each core emits
half of its pair's output rows.
"""

import numpy as np
import ml_dtypes

import concourse.bacc as bacc
import concourse.bass as bass
import concourse.mybir as mybir
from concourse.tile import TileContext
from concourse.bass_utils import run_bass_kernel_spmd

B, S, D, H = 4, 2048, 1024, 16
HL = 8          # heads per core
DK = 64         # head dim
NCORES = 8
DT = D // 128   # 8 d-tiles (contraction tiles)
OT = HL * DK // 128   # 4 o-tiles for Q^T/K^T (2 heads per tile)
ST = S // 128   # 16 s-tiles
NCH = S // 512  # 4 sequence chunks of 512
VW = DK + 1     # V columns per head incl. ones column

BF16 = mybir.dt.bfloat16
F32 = mybir.dt.float32
NEG = -1.0e9

_compiled = {}


def _build_nc():
    nc = bacc.Bacc("TRN2", target_bir_lowering=False, debug=False,
                   num_devices=NCORES)

    xT = nc.dram_tensor("xT", [D, S], BF16, kind="ExternalInput")
    wqT = nc.dram_tensor("wqT", [D, HL * DK], BF16, kind="ExternalInput")
    wkT = nc.dram_tensor("wkT", [D, HL * DK], BF16, kind="ExternalInput")
    wvT = nc.dram_tensor("wvT", [D, HL * DK], BF16, kind="ExternalInput")
    woT = nc.dram_tensor("woT", [HL * DK, D], BF16, kind="ExternalInput")
    cosT = nc.dram_tensor("cosT", [128, S], BF16, kind="ExternalInput")
    sinT = nc.dram_tensor("sinT", [128, S], BF16, kind="ExternalInput")
    swapT = nc.dram_tensor("swapT", [128, 128], BF16, kind="ExternalInput")
    maskT = nc.dram_tensor("maskT", [128, 128], F32, kind="ExternalInput")
    # each core outputs half of its pair's rows (ReduceScatter halves)
    y = nc.dram_tensor("y", [S // 2, D], BF16, kind="ExternalOutput")
    ypart = nc.dram_tensor("ypart", [S, D], BF16)
    yred = nc.dram_tensor("yred", [S // 2, D], BF16)

    groups = [[0, 1], [2, 3], [4, 5], [6, 7]]

    with TileContext(nc) as tc:
        with (
            tc.tile_pool(name="big", bufs=1) as big,
            tc.tile_pool(name="work", bufs=3) as work,
            tc.tile_pool(name="ptile", bufs=10) as ptile,
            tc.tile_pool(name="norm", bufs=2) as normp,
            tc.tile_pool(name="ps_s", bufs=3, space="PSUM") as ps_s,
            tc.tile_pool(name="ps_o", bufs=1, space="PSUM") as ps_o,
        ):
            # ---- inputs; ordered so chunk-0 work can start early ----
            w_sb = {"q": [], "k": [], "v": []}
            for k in range(DT):
                t = big.tile([128, HL * DK], BF16, tag=f"wq{k}", name=f"wq{k}")
                nc.scalar.dma_start(out=t[:], in_=wqT[128 * k:128 * (k + 1), :])
                w_sb["q"].append(t)
            mask_sb = big.tile([128, 128], F32, tag="mask")
            nc.scalar.dma_start(out=mask_sb[:], in_=maskT[:])
            cos_sb = big.tile([128, S], BF16, tag="cos")
            nc.scalar.dma_start(out=cos_sb[:, 0:512], in_=cosT[:, 0:512])
            sin_sb = big.tile([128, S], BF16, tag="sin")
            nc.scalar.dma_start(out=sin_sb[:, 0:512], in_=sinT[:, 0:512])
            swap_sb = big.tile([128, 128], BF16, tag="swap")
            nc.scalar.dma_start(out=swap_sb[:], in_=swapT[:])
            xT_sb = [big.tile([128, S], BF16, tag=f"xT{k}", name=f"xT{k}")
                     for k in range(DT)]
            for k in range(DT):
                nc.sync.dma_start(out=xT_sb[k][:, 0:512],
                                  in_=xT[128 * k:128 * (k + 1), 0:512])
            for k in range(DT):
                t = big.tile([128, HL * DK], BF16, tag=f"wk{k}", name=f"wk{k}")
                nc.gpsimd.dma_start(out=t[:], in_=wkT[128 * k:128 * (k + 1), :])
                w_sb["k"].append(t)
            for k in range(DT):
                t = big.tile([128, HL * DK], BF16, tag=f"wv{k}", name=f"wv{k}")
                nc.gpsimd.dma_start(out=t[:], in_=wvT[128 * k:128 * (k + 1), :])
                w_sb["v"].append(t)
            nc.gpsimd.dma_start(out=cos_sb[:, 512:1024], in_=cosT[:, 512:1024])
            nc.gpsimd.dma_start(out=sin_sb[:, 512:1024], in_=sinT[:, 512:1024])
            woT_sb = []
            for k in range(OT):
                t = big.tile([128, D], BF16, tag=f"wo{k}", name=f"wo{k}")
                nc.gpsimd.dma_start(out=t[:], in_=woT[128 * k:128 * (k + 1), :])
                woT_sb.append(t)
            nc.gpsimd.dma_start(out=cos_sb[:, 1024:S], in_=cosT[:, 1024:S])
            nc.gpsimd.dma_start(out=sin_sb[:, 1024:S], in_=sinT[:, 1024:S])

            # ---- persistent activations (per-chunk tiles) ----
            qrot_sb = [[big.tile([128, 512], BF16, tag=f"qr{m}_{t}",
                                 name=f"qr{m}_{t}") for t in range(OT)]
                       for m in range(NCH)]
            krot_sb = [[big.tile([128, 512], BF16, tag=f"kr{m}_{t}",
                                 name=f"kr{m}_{t}") for t in range(OT)]
                       for m in range(NCH)]
            vaug_sb = [big.tile([128, HL * VW], BF16, tag=f"va{st}",
                                name=f"va{st}") for st in range(ST)]
            for st in range(ST):
                ones = vaug_sb[st][:].rearrange(
                    "p (h d) -> p h d", d=VW)[:, :, DK:VW]
                nc.gpsimd.memset(ones, 1.0)
            oT_sb = [[big.tile([128, 512], BF16, tag=f"oT{m}_{t}",
                               name=f"oT{m}_{t}") for t in range(OT)]
                     for m in range(NCH)]

            def v_unit(m, st):
                ps = ps_s.tile([128, 1024], F32, tag="sps", name="psv")
                for k in range(DT):
                    nc.tensor.matmul(
                        ps[:, 0:512],
                        lhsT=xT_sb[k][:, 128 * st:128 * (st + 1)],
                        rhs=w_sb["v"][k][:],
                        start=(k == 0), stop=(k == DT - 1),
                    )
                dst = vaug_sb[st][:].rearrange(
                    "p (h d) -> p h d", d=VW)[:, :, 0:DK]
                src = ps[:, 0:512].rearrange("p (h d) -> p h d", d=DK)
                nc.vector.tensor_copy(dst, src)

            def qk_unit(wname, m, t):
                sl = slice(512 * m, 512 * (m + 1))
                rot = (qrot_sb if wname == "q" else krot_sb)[m][t]
                ps = ps_s.tile([128, 1024], F32, tag="sps", name="ps")
                for k in range(DT):
                    nc.tensor.matmul(
                        ps[:, 0:512],
                        lhsT=w_sb[wname][k][:, 128 * t:128 * (t + 1)],
                        rhs=xT_sb[k][:, sl],
                        start=(k == 0), stop=(k == DT - 1),
                    )
                raw = work.tile([128, 512], BF16, tag="raw")
                nc.vector.tensor_copy(raw[:], ps[:, 0:512])
                nc.tensor.matmul(ps[:, 512:1024], lhsT=swap_sb[:],
                                 rhs=raw[:], start=True, stop=True)
                t1 = work.tile([128, 512], BF16, tag="t1")
                nc.vector.tensor_mul(t1[:], raw[:], cos_sb[:, sl])
                t2 = work.tile([128, 512], BF16, tag="t2")
                nc.vector.tensor_mul(t2[:], ps[:, 512:1024], sin_sb[:, sl])
                nc.vector.tensor_add(rot[:], t1[:], t2[:])

            def attn_tp(m, tp, sums8, osb):
                i0 = 512 * m
                njb = 4 * m + 4
                o_pse = ps_o.tile([VW, 512], F32, tag="pse", bufs=1)
                o_pso = ps_o.tile([VW, 512], F32, tag="pso", bufs=1)
                pTs = []
                for jb in range(njb):
                    j0 = 128 * jb
                    dlt = max(0, j0 - i0)
                    kr = krot_sb[jb // 4][tp]
                    jl = j0 % 512
                    s_ps = ps_s.tile([128, 1024], F32, tag="sps", name="sps")
                    for half, po in ((0, 0), (1, DK)):
                        nc.tensor.matmul(
                            s_ps[:, 512 * half + dlt:512 * (half + 1)],
                            lhsT=kr[po:po + DK, jl:jl + 128],
                            rhs=qrot_sb[m][tp][po:po + DK, dlt:512],
                            start=True, stop=True,
                        )
                    if j0 >= i0:
                        s3 = s_ps[:].rearrange("p (b f) -> p b f", b=2)
                        nc.vector.tensor_add(
                            s3[:, :, dlt:dlt + 128],
                            s3[:, :, dlt:dlt + 128],
                            mask_sb[:].rearrange("p (b f) -> p b f", b=1)
                            .broadcast_to([128, 2, 128]))
                    pT = ptile.tile([128, 1024], BF16, tag="pT")
                    nc.scalar.activation(
                        pT[:].rearrange("p (b f) -> p b f", b=2)[:, :, dlt:512],
                        s_ps[:].rearrange("p (b f) -> p b f", b=2)[:, :, dlt:512],
                        mybir.ActivationFunctionType.Exp, scale=0.125)
                    pTs.append(pT)
                for jb in range(njb):
                    dlt = max(0, 128 * jb - i0)
                    for half, o_ps in ((0, o_pse), (1, o_pso)):
                        cs = VW * (2 * tp + half)
                        nc.tensor.matmul(
                            o_ps[:, dlt:512],
                            lhsT=vaug_sb[jb][:, cs:cs + VW],
                            rhs=pTs[jb][:, 512 * half + dlt:512 * (half + 1)],
                            start=(jb == 0), stop=(jb == njb - 1),
                        )
                for half, o_ps in ((0, o_pse), (1, o_pso)):
                    h = 2 * tp + half
                    ot = normp.tile([VW, 512], BF16, tag=f"osb{h}", bufs=1,
                                    name=f"osb{h}")
                    nc.vector.tensor_copy(ot[:], o_ps[:])
                    nc.sync.dma_start(out=sums8[tp // 2][h % 4:h % 4 + 1, :],
                                      in_=ot[DK:VW, :])
                    osb[h] = ot

            def norm_half(m, hf, sums8, osb):
                s32 = normp.tile([4, 512], F32, tag="s32", bufs=2, name="s32")
                nc.vector.tensor_copy(s32[:], sums8[hf][:])
                rec = normp.tile([4, 512], F32, tag="rec", bufs=2, name="rec")
                nc.vector.reciprocal_approx_fast(rec[:], s32[:])
                recb = normp.tile([4, 512], BF16, tag="recb", bufs=2,
                                  name="recb")
                nc.vector.tensor_copy(recb[:], rec[:])
                stage = normp.tile([1, 4 * 512], BF16, tag="stage", bufs=1,
                                   name="stage")
                nc.scalar.dma_start(out=stage[:], in_=recb[:])
                rep = normp.tile([DK, 4 * 512], BF16, tag="rep", bufs=1,
                                 name="rep")
                nc.gpsimd.partition_broadcast(rep[:], stage[:])
                for hh in range(4):
                    h = 4 * hf + hh
                    tp2, po = h // 2, DK * (h % 2)
                    nc.vector.tensor_mul(
                        oT_sb[m][tp2][po:po + DK, :], osb[h][0:DK, :],
                        rep[:, 512 * hh:512 * (hh + 1)])

            def proj_r(m, r2, ych_eng=None):
                r0 = 512 * m + 128 * r2
                yp = ps_s.tile([128, 1024], F32, tag="sps", name="yp")
                for nn in range(2):
                    for k in range(OT):
                        nc.tensor.matmul(
                            yp[:, 512 * nn:512 * (nn + 1)],
                            lhsT=oT_sb[m][k][:, 128 * r2:128 * (r2 + 1)],
                            rhs=woT_sb[k][:, 512 * nn:512 * (nn + 1)],
                            start=(k == 0), stop=(k == OT - 1),
                        )
                ych = work.tile([128, 1024], BF16, tag="ych")
                if ych_eng is None:
                    nc.vector.tensor_copy(ych[:], yp[:])
                else:
                    ych_eng.copy(out=ych[:], in_=yp[:])
                nc.sync.dma_start(out=ypart[r0:r0 + 128, :], in_=ych[:])

            def rs_chunk(m):
                q0 = 512 * m
                nc.gpsimd.collective_compute(
                    "ReduceScatter", mybir.AluOpType.add, replica_groups=groups,
                    ins=[ypart[q0:q0 + 512, :].opt()],
                    outs=[yred[256 * m:256 * (m + 1), :].opt()],
                )

            def y_dma(m):
                nc.sync.dma_start(out=y[256 * m:256 * (m + 1), :],
                                  in_=yred[256 * m:256 * (m + 1), :])

            # ---- pipelined emission ----
            # prep units per chunk: first two Q/K pairs, V units, last pairs
            prep = []
            for m in range(NCH):
                for t in (0, 1):
                    prep.append(("q", m, t))
                    prep.append(("k", m, t))
                for st in range(4 * m, 4 * m + 4):
                    prep.append(("v", m, st))
                for t in (2, 3):
                    prep.append(("q", m, t))
                    prep.append(("k", m, t))

            def emit_prep(upto):
                while emit_prep.cursor < min(upto, len(prep)):
                    kind, m, i = prep[emit_prep.cursor]
                    if kind == "v":
                        v_unit(m, i)
                    else:
                        qk_unit(kind, m, i)
                    emit_prep.cursor += 1
            emit_prep.cursor = 0

            def prefix_needed(m, tp):
                # attn(m, tp) needs all of chunk m's V plus Q/K pairs <= tp
                return 12 * m + {0: 8, 1: 8, 2: 10, 3: 12}[tp]

            sums8 = osb = None
            emit_prep(prefix_needed(0, 0))
            for m in range(NCH):
                sums8 = [normp.tile([4, 512], BF16, tag=f"sums{hf}", bufs=2,
                                    name=f"sums{hf}") for hf in range(2)]
                osb = [None] * HL
                for tp in range(OT):
                    attn_tp(m, tp, sums8, osb)
                    nxt = (m, tp + 1) if tp < 3 else (m + 1, 0)
                    if nxt[0] < NCH:
                        emit_prep(prefix_needed(*nxt))
                    if m >= 1 and tp < 2:
                        proj_r(m - 1, 2 * tp)
                        proj_r(m - 1, 2 * tp + 1)
                    if m >= 1 and tp == 2:
                        rs_chunk(m - 1)
                    if m == 0 and tp == 0:
                        for k in range(DT):
                            nc.sync.dma_start(
                                out=xT_sb[k][:, 512:1024],
                                in_=xT[128 * k:128 * (k + 1), 512:1024])
                    if m == 0 and tp == 2:
                        for k in range(DT):
                            nc.sync.dma_start(
                                out=xT_sb[k][:, 1024:S],
                                in_=xT[128 * k:128 * (k + 1), 1024:S])
                    if tp == 1:
                        norm_half(m, 0, sums8, osb)
                norm_half(m, 1, sums8, osb)
            for r2 in range(4):
                proj_r(NCH - 1, r2, ych_eng=nc.scalar)
            rs_chunk(NCH - 1)
            for m in range(NCH):
                y_dma(m)

    nc.compile()
    return nc


def _prep_inputs(x, Wq, Wk, Wv, Wo, cos_emb, sin_emb, token_positions):
    bf = ml_dtypes.bfloat16
    cos_g = np.asarray(cos_emb)[np.asarray(token_positions)]  # [S, DK]
    sin_g = np.asarray(sin_emb)[np.asarray(token_positions)]
    # [128, S]: partition p -> head-dim p % 64
    cosT = np.ascontiguousarray(np.tile(cos_g.T, (2, 1))).astype(bf)
    sinT = np.ascontiguousarray(np.tile(sin_g.T, (2, 1))).astype(bf)
    # rotate-half-interleaved as a matmul: rh = SWAP @ q (per 128-dim tile)
    swap = np.zeros((128, 128), np.float32)
    for j in range(64):
        swap[2 * j, 2 * j + 1] = -1.0
        swap[2 * j + 1, 2 * j] = 1.0
    swapT = np.ascontiguousarray(swap.T).astype(bf)
    # causal mask for the diagonal 128x128 block in S^T=[j,i] layout
    jj = np.arange(128)[:, None]
    ii = np.arange(128)[None, :]
    maskT = np.where(ii >= jj, 0.0, NEG).astype(np.float32)

    in_maps = []
    for c in range(NCORES):
        b, hh = c // 2, c % 2
        cols = slice(512 * hh, 512 * (hh + 1))
        in_maps.append({
            "xT": np.ascontiguousarray(np.asarray(x)[b].T).astype(bf),
            "wqT": np.ascontiguousarray(np.asarray(Wq)[cols, :].T).astype(bf),
            "wkT": np.ascontiguousarray(np.asarray(Wk)[cols, :].T).astype(bf),
            "wvT": np.ascontiguousarray(np.asarray(Wv)[cols, :].T).astype(bf),
            "woT": np.ascontiguousarray(np.asarray(Wo)[:, cols].T).astype(bf),
            "cosT": cosT, "sinT": sinT, "swapT": swapT, "maskT": maskT,
        })
    return in_maps


def kernel(x, Wq, Wk, Wv, Wo, cos_emb, sin_emb, token_positions, **run_kwargs):
    if "nc" not in _compiled:
        _compiled["nc"] = _build_nc()
    nc = _compiled["nc"]
    in_maps = _prep_inputs(x, Wq, Wk, Wv, Wo, cos_emb, sin_emb, token_positions)
    res = run_bass_kernel_spmd(nc, in_maps, list(range(NCORES)), **run_kwargs)
    # reassemble: per pair, chunk m rows [512m:512m+256) from the even core,
    # [512m+256:512m+512) from the odd core (ReduceScatter rank order)
    out = np.empty((B, S, D), np.float32)
    for b in range(B):
        for r in range(2):
            yh = np.asarray(res.results[2 * b + r]["y"]).astype(np.float32)
            for m in range(NCH):
                out[b, 512 * m + 256 * r:512 * m + 256 * (r + 1)] = \
                    yh[256 * m:256 * (m + 1)]
    if run_kwargs:
        kernel.last_result = res
    return out
